# revision 2
# baseline (speedup 1.0000x reference)
"""Trainium2 Bass kernel for nn_CrossAttention (dual cross-attention).

Problem (hardcoded shapes): x [1, 256, 8, 32, 32] fp32, C=256, C8=32,
N = 8*32*32 = 8192.

    QA/KA/QB/KB = 1x1 conv (C->32), VA/VB = 1x1 conv (C->256)
    attn_BA = softmax_m(QB^T KA);  F_B = VA @ attn_BA^T
    attn_AB = softmax_m(QA^T KB);  F_A = VB @ attn_AB^T
    out1 = gamma_A*F_A + x ; out2 = gamma_B*F_B + x

Sharding: sequence-parallel over query positions n across 8 cores
(1024 rows each); K/V projections over the full sequence are computed
redundantly per core (cheap relative to attention).

Per-core device algorithm (all matmuls bf16, fp32 accumulation):
  - K stored partition-replicated x4 ([128, 8192]) so the K=32 logit
    matmuls can be packed 4-per-PE-pass with tile_position row tiling.
  - S^T tiles [m,n] produced directly in transposed layout; softmax max-
    subtraction is skipped (logits are O(3), exp is safe) so softmax
    needs only exp + row sums.
  - V^T ([m, c]) is augmented with a ones column; the attention matmul
    F^T[n, 0:256] = sum_m P^T[m,n] V^T[m,c] then yields row sums in
    column 256 for free -> normalization is a per-partition scalar
    multiply after the matmul.
  - F^T is normalized, transposed back on the PE, scaled by gamma, and
    added to the residual.
"""

import numpy as np
import ml_dtypes

import concourse.bass as bass
import concourse.mybir as mybir
import concourse.tile as tile
from concourse.bass_utils import run_bass_kernel_spmd
from concourse.masks import make_identity

F32 = mybir.dt.float32
BF16 = mybir.dt.bfloat16
BF = ml_dtypes.bfloat16

C = 256
C8 = 32
N = 8192
NCORES = 8
NT = N // NCORES  # 1024 query rows per core
NN = 2  # n-chunks of 512 per core
NW = 512  # n-chunk width
G = 16  # m super-chunks of 512
ALU = mybir.AluOpType


def _emit_body(nc, tc, ctx, tensors):
    """Emit one full forward pass. tensors: dict of dram handles."""
    import contextlib

    sb = ctx.enter_context(tc.tile_pool(name="sb", bufs=1))

    ident = sb.tile([128, 128], F32)
    make_identity(nc, ident)

    def load2(name, w, dt=BF16):
        t = sb.tile([128, 2, w], dt, name=name)
        nc.sync.dma_start(out=t, in_=tensors[name].rearrange("(a p) m -> p a m", p=128))
        return t

    # weights: [256, w] dram -> [128, 2, w] sbuf (c' on partitions)
    wqa = load2("wqa", 128)
    wqb = load2("wqb", 128)
    wka = load2("wka", 128)
    wkb = load2("wkb", 128)
    wva = load2("wva", 256)
    wvb = load2("wvb", 256)
    xq = load2("xq", NT)
    xres = load2("xres", NT, dt=F32)
    xs = load2("xb", N)

    small = {}
    for name, w in (
        ("bqa", 1), ("bqb", 1), ("bka", 1), ("bkb", 1),
        ("g1", 1), ("g2", 1), ("c1", 2), ("c2", 2),
    ):
        t = sb.tile([128, w], F32, name=name)
        nc.sync.dma_start(out=t, in_=tensors[name][:, :])
        small[name] = t

    qa_sb = sb.tile([128, NT], BF16)
    qb_sb = sb.tile([128, NT], BF16)
    ka_sb = sb.tile([128, N], BF16)
    kb_sb = sb.tile([128, N], BF16)
    vat_sb = sb.tile([128, 64, C + 1], BF16)
    vbt_sb = sb.tile([128, 64, C + 1], BF16)
    nc.vector.memset(vat_sb[:, :, 256], 1.0)
    nc.vector.memset(vbt_sb[:, :, 256], 1.0)

    # ---- projections ----
    with tc.tile_pool(name="pps", bufs=4, space="PSUM") as pps:
        for w_sb, b_sb, q_sb in ((wqb, small["bqb"], qb_sb), (wqa, small["bqa"], qa_sb)):
            for n5 in range(NT // 512):
                qp = pps.tile([128, 512], F32, tag="pp", name="qp")
                for j in range(2):
                    nc.tensor.matmul(
                        qp, lhsT=w_sb[:, j, :], rhs=xq[:, j, n5 * 512 : (n5 + 1) * 512],
                        start=(j == 0), stop=(j == 1),
                    )
                nc.vector.tensor_scalar_add(
                    q_sb[:, n5 * 512 : (n5 + 1) * 512], in0=qp, scalar1=b_sb
                )
        for w_sb, b_sb, k_sb in ((wka, small["bka"], ka_sb), (wkb, small["bkb"], kb_sb)):
            for m5 in range(16):
                kp = pps.tile([128, 512], F32, tag="pp", name="kp")
                for j in range(2):
                    nc.tensor.matmul(
                        kp, lhsT=w_sb[:, j, :], rhs=xs[:, j, m5 * 512 : (m5 + 1) * 512],
                        start=(j == 0), stop=(j == 1),
                    )
                nc.vector.tensor_scalar_add(
                    k_sb[:, m5 * 512 : (m5 + 1) * 512], in0=kp, scalar1=b_sb
                )
        for w_sb, v_sb in ((wva, vat_sb), (wvb, vbt_sb)):
            for mc in range(64):
                vp = pps.tile([128, 512], F32, tag="pp", name="vp")
                for j in range(2):
                    nc.tensor.matmul(
                        vp[:, 0:256], lhsT=xs[:, j, mc * 128 : (mc + 1) * 128],
                        rhs=w_sb[:, j, :], start=(j == 0), stop=(j == 1),
                    )
                nc.vector.tensor_copy(out=v_sb[:, mc, 0:256], in_=vp[:, 0:256])

    # ---- attention ----
    tasks = (
        # (Q, K, V, gamma, cvec, out)  : task2 = BA, task1 = AB
        (qb_sb, ka_sb, vat_sb, small["g2"], small["c2"], tensors["out2"]),
        (qa_sb, kb_sb, vbt_sb, small["g1"], small["c1"], tensors["out1"]),
    )
    with (
        tc.tile_pool(name="spool", bufs=1, space="PSUM") as spool,
        tc.tile_pool(name="fpool", bufs=4, space="PSUM") as fpool,
        tc.tile_pool(name="ppool", bufs=2) as ppool,
        tc.tile_pool(name="fin", bufs=3) as fin,
    ):
        for q_sb, k_sb, v_sb, g_sb, c_sb, out_t in tasks:
            for nn in range(NN):
                n0 = nn * NW
                fs = [fpool.tile([128, C + 1], F32, tag="f", name="f") for _ in range(4)]
                pbs = [None] * G

                def emit_s(g, q_sb=q_sb, k_sb=k_sb, n0=n0, pbs=pbs):
                    sp = spool.tile([128, 4, 512], F32, tag="s", name="sp")
                    for i in range(4):
                        m0 = g * 512 + i * 128
                        nc.tensor.matmul(
                            sp[:, i, :],
                            lhsT=k_sb[32 * i : 32 * (i + 1), m0 : m0 + 128],
                            rhs=q_sb[32 * i : 32 * (i + 1), n0 : n0 + NW],
                            start=True, stop=True, tile_position=(32 * i, 0),
                        )
                    pb = ppool.tile([128, 4, NW], BF16, tag="p", name="pb")
                    nc.scalar.activation(pb, sp, mybir.ActivationFunctionType.Exp)
                    pbs[g] = pb

                def emit_av(g, v_sb=v_sb, fs=fs, pbs=pbs):
                    pb = pbs[g]
                    for i in range(4):
                        for k in range(4):
                            nc.tensor.matmul(
                                fs[k],
                                lhsT=pb[:, i, k * 128 : (k + 1) * 128],
                                rhs=v_sb[:, g * 4 + i, :],
                                start=(g == 0 and i == 0),
                                stop=(g == G - 1 and i == 3),
                            )

                # software-pipelined emission: S one step ahead of AV
                emit_s(0)
                for g in range(1, G):
                    emit_s(g)
                    emit_av(g - 1)
                emit_av(G - 1)

                for k in range(4):
                    rr = fin.tile([128, 1], F32, tag="r", name="rr")
                    nc.vector.reciprocal(rr, fs[k][:, 256:257])
                    fn = fin.tile([128, 256], F32, tag="fn", name="fn")
                    nc.vector.tensor_scalar_mul(fn, in0=fs[k][:, 0:256], scalar1=rr)
                    for cc in range(2):
                        tp = fpool.tile([128, 128], F32, tag="f", name="tp")
                        nc.tensor.transpose(tp, fn[:, 128 * cc : 128 * (cc + 1)], ident)
                        ob = fin.tile([128, 128], F32, tag="ob", name="ob")
                        nc.vector.tensor_scalar(
                            out=ob, in0=tp, scalar1=g_sb,
                            scalar2=c_sb[:, cc : cc + 1],
                            op0=ALU.mult, op1=ALU.add,
                        )
                        nc.vector.tensor_tensor(
                            out=ob, in0=ob,
                            in1=xres[:, cc, n0 + k * 128 : n0 + (k + 1) * 128],
                            op=ALU.add,
                        )
                        nc.sync.dma_start(
                            out=out_t[128 * cc : 128 * (cc + 1), n0 + k * 128 : n0 + (k + 1) * 128],
                            in_=ob,
                        )


def split_excess_waits(nc, max_waits=1):
    """The walrus build in this container rejects >1 sync wait per
    instruction; move excess waits onto preceding same-engine NOPs."""
    for f in nc.m.functions:
        for blk in f.blocks:
            insts = blk.instructions
            out = []
            changed = False
            for inst in insts:
                si = inst.sync_info
                waits = list(si.on_wait) if si is not None and si.on_wait else []
                if len(waits) > max_waits:
                    changed = True
                    head, rest = waits[:-max_waits], waits[-max_waits:]
                    while head:
                        chunk, head = head[:max_waits], head[max_waits:]
                        nop = mybir.InstNoOp(name=f"wsplit-{nc.next_id()}", ins=[], outs=[])
                        nop.engine = inst.engine
                        nop.sync_info = mybir.SyncInfo(on_wait=chunk, on_update=[])
                        out.append(nop)
                    inst.sync_info = mybir.SyncInfo(
                        on_wait=rest, on_update=list(si.on_update or [])
                    )
                out.append(inst)
            if changed:
                blk.instructions = out


def build_kernel(reps=1, split_waits=True):
    from contextlib import ExitStack

    nc = bass.Bass()
    tensors = {}
    specs = [
        ("xb", [C, N], BF16), ("xq", [C, NT], BF16), ("xres", [C, NT], F32),
        ("wqa", [C, 128], BF16), ("wqb", [C, 128], BF16),
        ("wka", [C, 128], BF16), ("wkb", [C, 128], BF16),
        ("wva", [C, C], BF16), ("wvb", [C, C], BF16),
        ("bqa", [128, 1], F32), ("bqb", [128, 1], F32),
        ("bka", [128, 1], F32), ("bkb", [128, 1], F32),
        ("g1", [128, 1], F32), ("g2", [128, 1], F32),
        ("c1", [128, 2], F32), ("c2", [128, 2], F32),
    ]
    for name, shape, dt in specs:
        tensors[name] = nc.dram_tensor(name, shape, dt, kind="ExternalInput")
    tensors["out1"] = nc.dram_tensor("out1", [C, NT], F32, kind="ExternalOutput")
    tensors["out2"] = nc.dram_tensor("out2", [C, NT], F32, kind="ExternalOutput")

    with tile.TileContext(nc) as tc, ExitStack() as ctx:
        if reps == 1:
            _emit_body(nc, tc, ctx, tensors)
        else:
            with tc.For_i(0, reps, 1):
                from contextlib import ExitStack as ES

                with ES() as ctx2:
                    _emit_body(nc, tc, ctx2, tensors)
    if split_waits:
        split_excess_waits(nc)
    return nc


def make_in_maps(inputs):
    """Host-side input marshaling: full inputs dict -> per-core in_maps."""
    x = np.asarray(inputs["x"], np.float32)
    xf = np.ascontiguousarray(x.reshape(C, N))
    xb = xf.astype(BF)

    def t(a):
        return np.ascontiguousarray(np.asarray(a, np.float32).T).astype(BF)

    wqa = np.ascontiguousarray(np.tile(t(inputs["WqA"]), (1, 4)))
    wqb = np.ascontiguousarray(np.tile(t(inputs["WqB"]), (1, 4)))
    wka = np.ascontiguousarray(np.tile(t(inputs["WkA"]), (1, 4)))
    wkb = np.ascontiguousarray(np.tile(t(inputs["WkB"]), (1, 4)))
    wva = t(inputs["WvA"])
    wvb = t(inputs["WvB"])

    def rep4(b):
        return np.ascontiguousarray(np.tile(np.asarray(b, np.float32), 4)[:, None])

    bqa, bqb = rep4(inputs["bqA"]), rep4(inputs["bqB"])
    bka, bkb = rep4(inputs["bkA"]), rep4(inputs["bkB"])
    gA = float(np.asarray(inputs["gamma_A"]).reshape(-1)[0])
    gB = float(np.asarray(inputs["gamma_B"]).reshape(-1)[0])
    g1 = np.full((128, 1), gA, np.float32)
    g2 = np.full((128, 1), gB, np.float32)
    # out1 = gamma_A * (VB-attn) + x : bias vec = gamma_A * bvB
    c1 = np.ascontiguousarray(
        (gA * np.asarray(inputs["bvB"], np.float32)).reshape(2, 128).T
    )
    c2 = np.ascontiguousarray(
        (gB * np.asarray(inputs["bvA"], np.float32)).reshape(2, 128).T
    )

    shared = dict(
        xb=xb, wqa=wqa, wqb=wqb, wka=wka, wkb=wkb, wva=wva, wvb=wvb,
        bqa=bqa, bqb=bqb, bka=bka, bkb=bkb, g1=g1, g2=g2, c1=c1, c2=c2,
    )
    in_maps = []
    for core in range(NCORES):
        sl = slice(core * NT, (core + 1) * NT)
        m = dict(shared)
        m["xq"] = np.ascontiguousarray(xb[:, sl])
        m["xres"] = np.ascontiguousarray(xf[:, sl])
        in_maps.append(m)
    return in_maps


def assemble_outputs(results):
    out1 = np.concatenate([r["out1"] for r in results], axis=1)
    out2 = np.concatenate([r["out2"] for r in results], axis=1)
    shape = (1, C, 8, 32, 32)
    return (
        np.ascontiguousarray(out1.reshape(shape)),
        np.ascontiguousarray(out2.reshape(shape)),
    )


_NC_CACHE = {}


def kernel(**inputs):
    if "nc" not in _NC_CACHE:
        _NC_CACHE["nc"] = build_kernel(reps=1)
    nc = _NC_CACHE["nc"]
    in_maps = make_in_maps(inputs)
    res = run_bass_kernel_spmd(nc, in_maps, core_ids=list(range(NCORES)))
    return assemble_outputs(res.results)
